# revision 1
# baseline (speedup 1.0000x reference)
"""Column-L2-normalization kernel for Trainium2 (8 NeuronCores, SPMD).

Computes y = x / sqrt(sum(x*x, axis=0)) for x of shape (524288, 256) fp32.

Strategy (row-sharded data parallel):
  - Each of the 8 cores gets a contiguous shard of 65536 rows (64 MB).
  - Pass 1: stream 2 MB tiles ([128 partitions x 4096 fp32], 16 rows per
    partition), square on ACT, strided reduce over the 16-row axis on DVE,
    accumulate per-(partition, column) sums.
  - Reduce over partitions with a ones-vector matmul, AllReduce the
    256-float per-column sums across the 8 cores, compute 1/sqrt.
  - Pass 2: re-stream the shard, multiply by the broadcast scale, write out.
"""

import numpy as np

import concourse.bacc as bacc
import concourse.mybir as mybir
from concourse import tile
from concourse import tile_utils
from concourse.bass_utils import run_bass_kernel_spmd

N_CORES = 8
M, C = 524288, 256
MLOC = M // N_CORES  # 65536 rows per core
P = 128  # SBUF partitions
R = 16  # rows per partition per tile
F = R * C  # free-dim elements per tile (4096)
T = MLOC // (P * R)  # tiles per core (32)
F32 = mybir.dt.float32


def build_nc():
    nc = bacc.Bacc("TRN2", target_bir_lowering=False, debug=False,
                   num_devices=N_CORES)
    x = nc.dram_tensor("x", [MLOC, C], F32, kind="ExternalInput")
    y = nc.dram_tensor("y", [MLOC, C], F32, kind="ExternalOutput")
    xt = x.ap().rearrange("(n p r) c -> n p (r c)", p=P, r=R)
    yt = y.ap().rearrange("(n p r) c -> n p (r c)", p=P, r=R)

    with tile.TileContext(nc) as tc:
        with (
            tc.tile_pool(name="xpool", bufs=8) as xpool,
            tc.tile_pool(name="small", bufs=2) as spool,
            tc.tile_pool(name="psum", bufs=2, space="PSUM") as ppool,
            tc.tile_pool(name="dram", bufs=1, space="DRAM") as dpool,
        ):
            acc = spool.tile([P, C], F32, tag="acc")
            nc.vector.memset(acc[:], 0.0)

            # ---- pass 1: per-column partial sum of squares ----
            for i in range(T):
                xtile = xpool.tile([P, F], F32, tag="x")
                nc.sync.dma_start(xtile[:], xt[i])
                nc.scalar.square(xtile[:], xtile[:])  # in place, ACT
                red = spool.tile([P, C], F32, tag="red")
                nc.vector.reduce_sum(
                    red[:],
                    xtile[:].rearrange("p (r c) -> p c r", c=C),
                    axis=mybir.AxisListType.X,
                )
                nc.vector.tensor_add(acc[:], acc[:], red[:])

            # ---- partition reduce + allreduce + rsqrt ----
            colsq = spool.tile([1, C], F32, tag="colsq")
            tile_utils.partition_sum(tc, colsq[:], acc[:])
            cin = dpool.tile([1, C], F32, tag="cin")
            cout = dpool.tile([1, C], F32, tag="cout")
            nc.sync.dma_start(cin[:], colsq[:])
            nc.gpsimd.collective_compute(
                "AllReduce",
                mybir.AluOpType.add,
                replica_groups=[list(range(N_CORES))],
                ins=[cin.opt()],
                outs=[cout.opt()],
            )
            gsum = spool.tile([1, C], F32, tag="gsum")
            nc.sync.dma_start(gsum[:], cout[:])
            rt = spool.tile([1, C], F32, tag="rt")
            nc.scalar.sqrt(rt[:], gsum[:])
            scl = spool.tile([1, C], F32, tag="scl")
            nc.vector.reciprocal(scl[:], rt[:])
            sclb = spool.tile([P, C], F32, tag="sclb")
            nc.gpsimd.partition_broadcast(sclb[:], scl[:])

            # ---- pass 2: scale and write out ----
            sclb3 = sclb[:].unsqueeze(1).broadcast_to((P, R, C))
            for i in range(T):
                xtile = xpool.tile([P, F], F32, tag="x")
                nc.sync.dma_start(xtile[:], xt[i])
                v = xtile[:].rearrange("p (r c) -> p r c", c=C)
                nc.vector.tensor_mul(v, v, sclb3)
                nc.sync.dma_start(yt[i], xtile[:])

    nc.compile()
    return nc


_NC_CACHE = None


def kernel(x) -> np.ndarray:
    global _NC_CACHE
    x = np.ascontiguousarray(np.asarray(x, dtype=np.float32))
    assert x.shape == (M, C)
    if _NC_CACHE is None:
        _NC_CACHE = build_nc()
    shards = x.reshape(N_CORES, MLOC, C)
    in_maps = [{"x": shards[i]} for i in range(N_CORES)]
    res = run_bass_kernel_spmd(_NC_CACHE, in_maps, list(range(N_CORES)))
    return np.concatenate([res.results[i]["y"] for i in range(N_CORES)], axis=0)


# revision 2
# speedup vs baseline: 1.0679x; 1.0679x over previous
"""Column-L2-normalization kernel for Trainium2 (8 NeuronCores, SPMD).

Computes y = x / sqrt(sum(x*x, axis=0)) for x of shape (524288, 256) fp32.

Strategy (row-sharded data parallel):
  - Each of the 8 cores gets a contiguous shard of 65536 rows (64 MB).
  - Pass 1: stream 2 MB tiles ([128 partitions x 4096 fp32], 16 rows per
    partition), square on ACT (bf16 out), reduce over partitions with a
    ones-vector matmul accumulating into PSUM across all tiles.
  - Reduce the 16-row axis on DVE, AllReduce the 256-float per-column
    sums across the 8 cores, compute 1/sqrt.
  - Pass 2: re-stream the shard, multiply by the broadcast scale, write out.
"""

import numpy as np

import concourse.bacc as bacc
import concourse.mybir as mybir
from concourse import tile
from concourse.bass_utils import run_bass_kernel_spmd

N_CORES = 8
M, C = 524288, 256
MLOC = M // N_CORES  # 65536 rows per core
P = 128  # SBUF partitions
R = 16  # rows per partition per tile
F = R * C  # free-dim elements per tile (4096)
T = MLOC // (P * R)  # tiles per core (32)
MM = 512  # moving free dim per matmul
F32 = mybir.dt.float32
BF16 = mybir.dt.bfloat16


def build_nc():
    nc = bacc.Bacc("TRN2", target_bir_lowering=False, debug=False,
                   num_devices=N_CORES)
    x = nc.dram_tensor("x", [MLOC, C], F32, kind="ExternalInput")
    y = nc.dram_tensor("y", [MLOC, C], F32, kind="ExternalOutput")
    xt = x.ap().rearrange("(n p r) c -> n p (r c)", p=P, r=R)
    yt = y.ap().rearrange("(n p r) c -> n p (r c)", p=P, r=R)

    with tile.TileContext(nc) as tc:
        with (
            tc.tile_pool(name="xpool", bufs=10) as xpool,
            tc.tile_pool(name="sqpool", bufs=2) as sqpool,
            tc.tile_pool(name="small", bufs=2) as spool,
            tc.tile_pool(name="psum", bufs=1, space="PSUM") as ppool,
            tc.tile_pool(name="dram", bufs=1, space="DRAM") as dpool,
        ):
            ones = spool.tile([P, 1], BF16, tag="ones")
            nc.vector.memset(ones[:], 1.0)
            ps = ppool.tile([1, F], F32, tag="ps")

            # ---- pass 1: per-(row-in-partition, column) sums of squares ----
            for i in range(T):
                xtile = xpool.tile([P, F], F32, tag="x")
                nc.sync.dma_start(xtile[:], xt[i])
                sq = sqpool.tile([P, F], BF16, tag="sq")
                nc.scalar.square(sq[:], xtile[:])
                for b in range(F // MM):
                    nc.tensor.matmul(
                        ps[:, b * MM:(b + 1) * MM],
                        ones[:],
                        sq[:, b * MM:(b + 1) * MM],
                        start=(i == 0),
                        stop=(i == T - 1),
                    )

            # ---- r-axis reduce + allreduce + rsqrt ----
            colsq = spool.tile([1, C], F32, tag="colsq")
            nc.vector.reduce_sum(
                colsq[:],
                ps[:].rearrange("p (r c) -> p c r", c=C),
                axis=mybir.AxisListType.X,
            )
            cin = dpool.tile([1, C], F32, tag="cin")
            cout = dpool.tile([1, C], F32, tag="cout")
            nc.sync.dma_start(cin[:], colsq[:])
            nc.gpsimd.collective_compute(
                "AllReduce",
                mybir.AluOpType.add,
                replica_groups=[list(range(N_CORES))],
                ins=[cin.opt()],
                outs=[cout.opt()],
            )
            gsum = spool.tile([1, C], F32, tag="gsum")
            nc.sync.dma_start(gsum[:], cout[:])
            rt = spool.tile([1, C], F32, tag="rt")
            nc.scalar.sqrt(rt[:], gsum[:])
            scl = spool.tile([1, C], F32, tag="scl")
            nc.vector.reciprocal(scl[:], rt[:])
            sclb = spool.tile([P, C], F32, tag="sclb")
            nc.gpsimd.partition_broadcast(sclb[:], scl[:])

            # ---- pass 2: scale and write out ----
            sclb3 = sclb[:].unsqueeze(1).broadcast_to((P, R, C))
            for i in range(T):
                xtile = xpool.tile([P, F], F32, tag="x")
                nc.sync.dma_start(xtile[:], xt[i])
                v = xtile[:].rearrange("p (r c) -> p r c", c=C)
                nc.vector.tensor_mul(v, v, sclb3)
                nc.sync.dma_start(yt[i], xtile[:])

    nc.compile()
    return nc


_NC_CACHE = None


def kernel(x) -> np.ndarray:
    global _NC_CACHE
    x = np.ascontiguousarray(np.asarray(x, dtype=np.float32))
    assert x.shape == (M, C)
    if _NC_CACHE is None:
        _NC_CACHE = build_nc()
    shards = x.reshape(N_CORES, MLOC, C)
    in_maps = [{"x": shards[i]} for i in range(N_CORES)]
    res = run_bass_kernel_spmd(_NC_CACHE, in_maps, list(range(N_CORES)))
    return np.concatenate([res.results[i]["y"] for i in range(N_CORES)], axis=0)


# revision 3
# speedup vs baseline: 1.1076x; 1.0372x over previous
"""Column-L2-normalization kernel for Trainium2 (8 NeuronCores, SPMD).

Computes y = x / sqrt(sum(x*x, axis=0)) for x of shape (524288, 256) fp32.

Strategy (row-sharded data parallel):
  - Each of the 8 cores gets a contiguous shard of 65536 rows (64 MB).
  - Pass 1: stream 1 MB tiles ([128 partitions x 2048 fp32], 8 rows per
    partition), square on ACT (bf16 out), fold once on DVE (fp32), reduce
    over partitions with a ones-vector matmul accumulating into PSUM.
  - Reduce the folded-row axis on DVE, AllReduce the 256-float per-column
    sums across the 8 cores, compute 1/sqrt.
  - Pass 2: re-stream the shard, multiply by the broadcast scale, write out.
    Deep load prefetch (20 tiles) hides the collective latency.
"""

import numpy as np

import concourse.bacc as bacc
import concourse.mybir as mybir
from concourse import tile
from concourse.bass_utils import run_bass_kernel_spmd

N_CORES = 8
M, C = 524288, 256
MLOC = M // N_CORES  # 65536 rows per core
P = 128  # SBUF partitions
R = 8  # rows per partition per tile
F = R * C  # free-dim elements per tile (2048)
H = F // 2  # folded free size (1024)
T = MLOC // (P * R)  # tiles per core (64)
MM = 512  # moving free dim per matmul
F32 = mybir.dt.float32
BF16 = mybir.dt.bfloat16
XBUFS = 20


def build_nc():
    nc = bacc.Bacc("TRN2", target_bir_lowering=False, debug=False,
                   num_devices=N_CORES)
    x = nc.dram_tensor("x", [MLOC, C], F32, kind="ExternalInput")
    y = nc.dram_tensor("y", [MLOC, C], F32, kind="ExternalOutput")
    xt = x.ap().rearrange("(n p r) c -> n p (r c)", p=P, r=R)
    yt = y.ap().rearrange("(n p r) c -> n p (r c)", p=P, r=R)

    with tile.TileContext(nc) as tc:
        with (
            tc.tile_pool(name="xpool", bufs=XBUFS) as xpool,
            tc.tile_pool(name="sqpool", bufs=2) as sqpool,
            tc.tile_pool(name="small", bufs=2) as spool,
            tc.tile_pool(name="psum", bufs=1, space="PSUM") as ppool,
            tc.tile_pool(name="dram", bufs=1, space="DRAM") as dpool,
        ):
            ones = spool.tile([P, 1], F32, tag="ones")
            nc.vector.memset(ones[:], 1.0)
            ps = ppool.tile([1, H], F32, tag="ps")

            # ---- pass 1: per-(folded-row, column) sums of squares ----
            for i in range(T):
                xtile = xpool.tile([P, F], F32, tag="x")
                nc.sync.dma_start(xtile[:], xt[i])
                sq = sqpool.tile([P, F], BF16, tag="sq")
                nc.scalar.square(sq[:], xtile[:])
                h = sqpool.tile([P, H], F32, tag="h")
                nc.vector.tensor_add(h[:], sq[:, :H], sq[:, H:])
                for b in range(H // MM):
                    nc.tensor.matmul(
                        ps[:, b * MM:(b + 1) * MM],
                        ones[:],
                        h[:, b * MM:(b + 1) * MM],
                        start=(i == 0),
                        stop=(i == T - 1),
                    )

            # ---- fold-axis reduce + allreduce + rsqrt ----
            colsq = spool.tile([1, C], F32, tag="colsq")
            nc.vector.reduce_sum(
                colsq[:],
                ps[:].rearrange("p (r c) -> p c r", c=C),
                axis=mybir.AxisListType.X,
            )
            cin = dpool.tile([1, C], F32, tag="cin")
            cout = dpool.tile([1, C], F32, tag="cout")
            nc.sync.dma_start(cin[:], colsq[:])
            nc.gpsimd.collective_compute(
                "AllReduce",
                mybir.AluOpType.add,
                replica_groups=[list(range(N_CORES))],
                ins=[cin.opt()],
                outs=[cout.opt()],
            )
            gsum = spool.tile([1, C], F32, tag="gsum")
            nc.sync.dma_start(gsum[:], cout[:])
            rt = spool.tile([1, C], F32, tag="rt")
            nc.scalar.sqrt(rt[:], gsum[:])
            scl = spool.tile([1, C], F32, tag="scl")
            nc.vector.reciprocal(scl[:], rt[:])
            sclb = spool.tile([P, C], F32, tag="sclb")
            nc.gpsimd.partition_broadcast(sclb[:], scl[:])

            # ---- pass 2: scale and write out ----
            sclb3 = sclb[:].unsqueeze(1).broadcast_to((P, R, C))
            for i in range(T):
                xtile = xpool.tile([P, F], F32, tag="x")
                nc.sync.dma_start(xtile[:], xt[i])
                v = xtile[:].rearrange("p (r c) -> p r c", c=C)
                nc.vector.tensor_mul(v, v, sclb3)
                nc.scalar.dma_start(yt[i], xtile[:])

    nc.compile()
    return nc


_NC_CACHE = None


def kernel(x) -> np.ndarray:
    global _NC_CACHE
    x = np.ascontiguousarray(np.asarray(x, dtype=np.float32))
    assert x.shape == (M, C)
    if _NC_CACHE is None:
        _NC_CACHE = build_nc()
    shards = x.reshape(N_CORES, MLOC, C)
    in_maps = [{"x": shards[i]} for i in range(N_CORES)]
    res = run_bass_kernel_spmd(_NC_CACHE, in_maps, list(range(N_CORES)))
    return np.concatenate([res.results[i]["y"] for i in range(N_CORES)], axis=0)


# revision 6
# speedup vs baseline: 1.1373x; 1.0268x over previous
"""Column-L2-normalization kernel for Trainium2 (8 NeuronCores, SPMD).

Computes y = x / sqrt(sum(x*x, axis=0)) for x of shape (524288, 256) fp32.

Strategy (row-sharded data parallel):
  - Each of the 8 cores gets a contiguous shard of 65536 rows (64 MB).
  - Pass 1: stream 1 MB tiles ([128 partitions x 2048 fp32], 8 rows per
    partition), square on ACT (bf16 out), fold once on DVE (fp32), reduce
    over partitions with a ones-vector matmul accumulating into PSUM.
  - Reduce the folded-row axis on DVE, AllReduce the 256-float per-column
    sums across the 8 cores, compute 1/sqrt.
  - Pass 2: re-stream the shard, multiply by the broadcast scale, write out.
    Deep load prefetch (20 tiles) hides the collective latency.
"""

import numpy as np

import concourse.bacc as bacc
import concourse.mybir as mybir
from concourse import tile
from concourse.bass_utils import run_bass_kernel_spmd

N_CORES = 8
M, C = 524288, 256
MLOC = M // N_CORES  # 65536 rows per core
P = 128  # SBUF partitions
R = 8  # rows per partition per tile
F = R * C  # free-dim elements per tile (2048)
H = F // 2  # folded free size (1024)
T = MLOC // (P * R)  # tiles per core (64)
MM = 512  # moving free dim per matmul
F32 = mybir.dt.float32
BF16 = mybir.dt.bfloat16
XBUFS = 20
NRES = 16  # tiles kept resident in SBUF between the passes


def build_nc():
    nc = bacc.Bacc("TRN2", target_bir_lowering=False, debug=False,
                   num_devices=N_CORES)
    x = nc.dram_tensor("x", [MLOC, C], F32, kind="ExternalInput")
    y = nc.dram_tensor("y", [MLOC, C], F32, kind="ExternalOutput")
    xt = x.ap().rearrange("(n p r) c -> n p (r c)", p=P, r=R)
    yt = y.ap().rearrange("(n p r) c -> n p (r c)", p=P, r=R)

    with tile.TileContext(nc) as tc:
        with (
            tc.tile_pool(name="xpool", bufs=XBUFS) as xpool,
            tc.tile_pool(name="sqpool", bufs=2) as sqpool,
            tc.tile_pool(name="small", bufs=2) as spool,
            tc.tile_pool(name="psum", bufs=1, space="PSUM") as ppool,
            tc.tile_pool(name="dram", bufs=1, space="DRAM") as dpool,
        ):
            ones = spool.tile([P, 1], F32, tag="ones")
            nc.vector.memset(ones[:], 1.0)
            ps = ppool.tile([1, H], F32, tag="ps")

            # ---- pass 1: per-(folded-row, column) sums of squares ----
            # The last NRES tiles stay resident in SBUF so pass 2 can skip
            # re-loading them (their squares go to a separate bf16 tile).
            resident = {}
            for i in range(T):
                xtile = xpool.tile([P, F], F32, tag="x")
                nc.sync.dma_start(xtile[:], xt[i])
                if i >= T - NRES:
                    resident[i] = xtile
                sq = sqpool.tile([P, F], BF16, tag="sq")
                nc.scalar.square(sq[:], xtile[:])
                h = sqpool.tile([P, H], F32, tag="h")
                nc.vector.tensor_add(h[:], sq[:, :H], sq[:, H:])
                for b in range(H // MM):
                    nc.tensor.matmul(
                        ps[:, b * MM:(b + 1) * MM],
                        ones[:],
                        h[:, b * MM:(b + 1) * MM],
                        start=(i == 0),
                        stop=(i == T - 1),
                    )

            # ---- fold-axis reduce + allreduce + rsqrt ----
            colsq = spool.tile([1, C], F32, tag="colsq")
            nc.vector.reduce_sum(
                colsq[:],
                ps[:].rearrange("p (r c) -> p c r", c=C),
                axis=mybir.AxisListType.X,
            )
            cin = dpool.tile([1, C], F32, tag="cin")
            cout = dpool.tile([1, C], F32, tag="cout")
            nc.sync.dma_start(cin[:], colsq[:])
            nc.gpsimd.collective_compute(
                "AllReduce",
                mybir.AluOpType.add,
                replica_groups=[list(range(N_CORES))],
                ins=[cin.opt()],
                outs=[cout.opt()],
            )
            gsum = spool.tile([1, C], F32, tag="gsum")
            nc.sync.dma_start(gsum[:], cout[:])
            rt = spool.tile([1, C], F32, tag="rt")
            nc.scalar.sqrt(rt[:], gsum[:])
            scl = spool.tile([1, C], F32, tag="scl")
            nc.vector.reciprocal(scl[:], rt[:])
            sclb = spool.tile([P, C], F32, tag="sclb")
            nc.gpsimd.partition_broadcast(sclb[:], scl[:])

            # ---- pass 2: scale and write out ----
            # Resident tiles first (no load needed), then re-stream the rest.
            sclb3 = sclb[:].unsqueeze(1).broadcast_to((P, R, C))
            order = list(range(T - NRES, T)) + list(range(T - NRES))
            for i in order:
                if i in resident:
                    xtile = resident[i]
                else:
                    xtile = xpool.tile([P, F], F32, tag="x")
                    nc.sync.dma_start(xtile[:], xt[i])
                v = xtile[:].rearrange("p (r c) -> p r c", c=C)
                nc.vector.tensor_mul(v, v, sclb3)
                nc.scalar.dma_start(yt[i], xtile[:])

    nc.compile()
    return nc


_NC_CACHE = None


def kernel(x) -> np.ndarray:
    global _NC_CACHE
    x = np.ascontiguousarray(np.asarray(x, dtype=np.float32))
    assert x.shape == (M, C)
    if _NC_CACHE is None:
        _NC_CACHE = build_nc()
    shards = x.reshape(N_CORES, MLOC, C)
    in_maps = [{"x": shards[i]} for i in range(N_CORES)]
    res = run_bass_kernel_spmd(_NC_CACHE, in_maps, list(range(N_CORES)))
    return np.concatenate([res.results[i]["y"] for i in range(N_CORES)], axis=0)
